# revision 2
# baseline (speedup 1.0000x reference)
"""Binarized 3x3 conv (XNOR-style): sign(conv2d(sign(x), sign(w)) + b).

Full-input contract: kernel(x=[32,256,56,56]f32, weight=[256,256,3,3]f32,
bias=[256]f32) -> [32,256,56,56]f32.

Strategy: data-parallel over batch across 8 NeuronCores (4 images/core).
Per core:
  - sign(x) encoded as +/-0.5 (exact: is_ge -> {0,1}, subtract 0.5) into a
    zero-padded 58x58 per-image layout, fp8e4 (or bf16).
  - sign(w) prepped on host as +/-1 in [c_partition, tap, (pair,) k] layout.
  - conv = 9 tap-shifted matmuls (fp8 DoubleRow, contract=256) accumulating
    into PSUM. All products are +/-0.5 with f32 accumulation, so
    psum == conv/2 exactly (conv is an even integer in [-2304, 2304]).
  - output sign = clamp(conv/2, -1, 1), exact for even integers including 0.
    One DVE tensor_scalar(min 1.0, max -1.0) per tile.
Bias is asserted zero (setup_inputs uses zeros); a nonzero bias falls back to
an exact 3-op sign path.
"""

import numpy as np

import concourse.bacc as bacc
import concourse.mybir as mybir
import concourse.tile as tile
from concourse.bass_utils import run_bass_kernel_spmd

N_CORES = 8
N_PER = 4          # images per core
C = 256            # input channels
K = 256            # output channels
H = W = 56
HP = WP = 58       # padded
XSP = HP * WP      # 3364 padded image pixels
HALF = 3376        # per-(image, pair-half) stride, padded to %16==0
RB = 8             # output rows per matmul tile
F = RB * WP        # 464 matmul free size (8 rows x 58, last 2 cols of each row garbage)
NBLK = H // RB     # 7 row blocks per image

USE_FP8 = True

_cache = {}


def _build(mode, with_bias):
    dt = mybir.dt
    xdt = dt.float8e4 if mode == "fp8" else dt.bfloat16
    nc = bacc.Bacc()
    x_d = nc.declare_dram_parameter("xs", [N_PER, C, H, W], dt.float32, isOutput=False)
    wfree = 9 * 2 * 256
    w_d = nc.declare_dram_parameter("wsgn", [128, wfree], xdt, isOutput=False)
    if with_bias:
        b_d = nc.declare_dram_parameter("bhalf", [128, 2], dt.float32, isOutput=False)
    o_d = nc.declare_dram_parameter("out", [N_PER, K, H, W], dt.float32, isOutput=True)

    with tile.TileContext(nc) as tc:
        with (
            tc.tile_pool(name="wpool", bufs=1) as wpool,
            tc.tile_pool(name="xsgn", bufs=N_PER) as xsgn_pool,
            tc.tile_pool(name="xf32", bufs=3) as xf_pool,
            tc.tile_pool(name="osb", bufs=6) as o_pool,
            tc.tile_pool(name="psum", bufs=8, space="PSUM") as p_pool,
        ):
            w_sb = wpool.tile([128, wfree], xdt)
            nc.sync.dma_start(w_sb[:], w_d[:])
            if with_bias:
                b_sb = wpool.tile([128, 2], dt.float32)
                nc.sync.dma_start(b_sb[:], b_d[:])

            # x sign tiles: one per image, both channel halves: [128, 2*HALF]
            # free index = ci*HALF + (y*58 + x) over the padded 58x58 grid.
            xs_tiles = []
            xv = x_d[:].rearrange("n c h w -> n c (h w)")
            for n in range(N_PER):
                xs = xsgn_pool.tile([128, 2 * HALF], xdt, tag="xsgn")
                nc.gpsimd.memset(xs[:], 0.0)
                xs_tiles.append(xs)
                for ci in range(2):
                    xf = xf_pool.tile([128, H * W], dt.float32, tag="xf32")
                    nc.sync.dma_start(xf[:], xv[n, ci * 128:(ci + 1) * 128, :])
                    dst = (
                        xs[:, ci * HALF: ci * HALF + XSP]
                        .rearrange("p (h w) -> p h w", h=HP)[:, 1:57, 1:57]
                    )
                    src = xf[:].rearrange("p (h w) -> p h w", h=H)
                    # (x>=0 -> {0,1}) - 0.5 = +/-0.5, exact
                    nc.vector.tensor_scalar(
                        dst, src, 0.0, 0.5, mybir.AluOpType.is_ge,
                        mybir.AluOpType.subtract,
                    )

            wv = w_sb[:].rearrange("p (t i k) -> p t i k", t=9, i=2)
            for kg in range(2):
                for n in range(N_PER):
                    xs = xs_tiles[n]
                    psums = [p_pool.tile([128, F], dt.float32, tag="ps", name=f"ps{kg}_{n}_{i}") for i in range(NBLK)]
                    if mode == "fp8":
                        xp = xs[:].rearrange("p (i f) -> p i f", i=2)
                        for tap in range(9):
                            ty, tx = tap // 3, tap % 3
                            lhsT = wv[:, tap, :, kg * 128:(kg + 1) * 128]
                            for rb in range(NBLK):
                                base = (rb * RB + ty) * WP + tx
                                rhs = xp[:, :, base: base + F]
                                nc.tensor.matmul(
                                    psums[rb][:], lhsT, rhs,
                                    start=(tap == 0), stop=(tap == 8),
                                    perf_mode=mybir.MatmulPerfMode.DoubleRow,
                                )
                    else:
                        step = 0
                        for ci in range(2):
                            for tap in range(9):
                                ty, tx = tap // 3, tap % 3
                                lhsT = wv[:, tap, ci, kg * 128:(kg + 1) * 128]
                                for rb in range(NBLK):
                                    base = ci * HALF + (rb * RB + ty) * WP + tx
                                    rhs = xs[:, base: base + F]
                                    nc.tensor.matmul(
                                        psums[rb][:], lhsT, rhs,
                                        start=(step == 0), stop=(step == 17),
                                    )
                                step += 1
                    for rb in range(NBLK):
                        osb = o_pool.tile([128, F], dt.float32, tag="osb")
                        if not with_bias:
                            # exact sign of even integers: clamp(v/2, -1, 1)
                            nc.vector.tensor_scalar(
                                osb[:], psums[rb][:], 1.0, -1.0,
                                mybir.AluOpType.min, mybir.AluOpType.max,
                            )
                        else:
                            # exact sign(v + b): (v/2+b/2 > 0) - (v/2+b/2 < 0)
                            tpos = o_pool.tile([128, F], dt.float32, tag="tpos")
                            tneg = o_pool.tile([128, F], dt.float32, tag="tneg")
                            bcol = b_sb[:, kg: kg + 1]
                            nc.vector.tensor_scalar(
                                tpos[:], psums[rb][:], bcol, 0.0,
                                mybir.AluOpType.add, mybir.AluOpType.is_gt,
                            )
                            nc.vector.tensor_scalar(
                                tneg[:], psums[rb][:], bcol, 0.0,
                                mybir.AluOpType.add, mybir.AluOpType.is_lt,
                            )
                            nc.vector.tensor_tensor(
                                osb[:], tpos[:], tneg[:], mybir.AluOpType.subtract,
                            )
                        src = osb[:].rearrange("p (r c) -> p r c", r=RB)[:, :, 0:W]
                        dst = o_d[n, kg * 128:(kg + 1) * 128, rb * RB: rb * RB + RB, :]
                        nc.sync.dma_start(dst, src)

    nc.finalize()
    return nc


def _prep_weights(weight, mode):
    dt = mybir.dt
    xdt = dt.float8e4 if mode == "fp8" else dt.bfloat16
    sgn = np.sign(weight.astype(np.float32))
    w4 = sgn.reshape(K, 2, 128, 3, 3)          # [k, i, p, ty, tx]
    arr = w4.transpose(2, 3, 4, 1, 0)          # [p, ty, tx, i, k]
    arr = np.ascontiguousarray(arr).reshape(128, 9 * 2 * 256)
    return arr.astype(mybir.dt.np(xdt))


def kernel(x, weight, bias, _profile=False, _trace_kwargs=None):
    mode = "fp8" if USE_FP8 else "bf16"
    x = np.asarray(x, dtype=np.float32)
    weight = np.asarray(weight, dtype=np.float32)
    bias = np.asarray(bias, dtype=np.float32)
    with_bias = bool(np.any(bias != 0.0))

    key = (mode, with_bias)
    if key not in _cache:
        _cache[key] = _build(mode, with_bias)
    nc = _cache[key]

    wsgn = _prep_weights(weight, mode)
    in_maps = []
    for c in range(N_CORES):
        m = {
            "xs": np.ascontiguousarray(x[c * N_PER:(c + 1) * N_PER]),
            "wsgn": wsgn,
        }
        if with_bias:
            m["bhalf"] = np.ascontiguousarray(
                (bias.reshape(2, 128).T * 0.5).astype(np.float32)
            )
        in_maps.append(m)

    res = run_bass_kernel_spmd(
        nc, in_maps, core_ids=list(range(N_CORES)),
        trace=_profile, **(_trace_kwargs or {}),
    )
    out = np.concatenate([res.results[c]["out"] for c in range(N_CORES)], axis=0)
    if _profile:
        kernel.last_exec_ns = res.exec_time_ns
        kernel.last_results = res
    return out
